# revision 6
# baseline (speedup 1.0000x reference)
import sys

if "/root/.axon_site/_ro/trn_rl_repo" not in sys.path:
    sys.path.insert(0, "/root/.axon_site/_ro/trn_rl_repo")

import numpy as np

B, S, D, H, DH = 16, 1024, 512, 8, 64
NCORES = 8
NB = B // NCORES  # batches per core
SCALE = D ** -0.5

_cache = {}


def _build():
    import concourse.bacc as bacc
    import concourse.tile as tile
    import concourse.mybir as mybir
    from concourse.masks import make_identity

    f32 = mybir.dt.float32
    f32r = mybir.dt.float32r
    bf16 = mybir.dt.bfloat16
    fp8 = mybir.dt.float8e4
    AF = mybir.ActivationFunctionType
    DR = mybir.MatmulPerfMode.DoubleRow

    nc = bacc.Bacc("TRN2", target_bir_lowering=False)
    X = nc.declare_dram_parameter("X", [NB, S, D], f32, isOutput=False)
    WQKV = nc.declare_dram_parameter("WQKV", [D, 3 * D], f32, isOutput=False)
    WPROJ = nc.declare_dram_parameter("WPROJ", [D, D], f32, isOutput=False)
    OUT = nc.declare_dram_parameter("OUT", [NB, S, D], f32, isOutput=True)

    with tile.TileContext(nc) as tc:
        with tc.tile_pool(name="sb", bufs=1) as sb, \
             tc.tile_pool(name="sbo", bufs=2) as sbo, \
             tc.tile_pool(name="sbw", bufs=2) as sbw, \
             tc.tile_pool(name="sbr", bufs=2) as sbr, \
             tc.tile_pool(name="pmi", bufs=2, space="PSUM") as pmi, \
             tc.tile_pool(name="psc", bufs=1, space="PSUM") as psc, \
             tc.tile_pool(name="pav", bufs=2, space="PSUM") as pav_pool:
            # persistent SBUF
            wq_sb = sb.tile([128, 4, D], f32r)
            wk_sb = sb.tile([128, 4, D], f32r)
            wv_sb = sb.tile([128, 4, D], f32r)
            wproj_f = sb.tile([128, 4, D], f32)
            wproj_b = sb.tile([128, 4, D], bf16)
            ident = sb.tile([128, 128], f32)
            x_tiles = [sb.tile([128, 8, D], f32r, name=f"x{b}") for b in range(NB)]
            xT = sb.tile([128, 4, S], f32r)
            # double-buffered (by batch parity) attention tensors
            qT8 = [sb.tile([128, 4, 2, S], fp8, name=f"q8_{p}") for p in range(2)]
            kT8 = [sb.tile([128, 4, 8, 2, 128], fp8, name=f"k8_{p}") for p in range(2)]
            vaug = [sb.tile([128, 8, H, 65], bf16, name=f"va_{p}") for p in range(2)]
            pt = [sb.tile([128, 8, 512], bf16, name=f"pt_{p}") for p in range(2)]
            ot = [sb.tile([128, 4, S], bf16, name=f"ot_{p}") for p in range(2)]
            out_sb = sb.tile([128, 8, D], f32)

            # startup DMAs: x halves and q/k weights interleaved on SP (in
            # consumption order); v + proj weights on ACT's queue (idle then).
            wqkv_split = WQKV[:].bitcast(f32r).rearrange("(t p) e -> p t e", p=128)
            x0_src = X[0].bitcast(f32r).rearrange("(t p) c -> p t c", p=128)
            nc.sync.dma_start(out=x_tiles[0][:, 0:4, :], in_=x0_src[:, 0:4, :])
            for j in range(4):
                for h in (2 * j, 2 * j + 1):
                    nc.sync.dma_start(out=wq_sb[:, :, h * 64:(h + 1) * 64],
                                      in_=wqkv_split[:, :, 192 * h:192 * h + 64])
                for h in (2 * j, 2 * j + 1):
                    nc.sync.dma_start(out=wk_sb[:, :, h * 64:(h + 1) * 64],
                                      in_=wqkv_split[:, :, 192 * h + 64:192 * h + 128])
                if j == 0:
                    nc.sync.dma_start(out=x_tiles[0][:, 4:8, :],
                                      in_=x0_src[:, 4:8, :])
            nc.sync.dma_start(
                out=x_tiles[1][:],
                in_=X[1].bitcast(f32r).rearrange("(t p) c -> p t c", p=128),
            )
            for h in range(H):
                nc.scalar.dma_start(
                    out=wv_sb[:, :, h * 64:(h + 1) * 64],
                    in_=wqkv_split[:, :, 192 * h + 128:192 * h + 192],
                )
            nc.scalar.dma_start(
                out=wproj_f[:],
                in_=WPROJ[:].rearrange("(t p) e -> p t e", p=128),
            )
            make_identity(nc, ident[:])
            identr = sb.tile([128, 128], f32r)
            identb = sb.tile([128, 128], bf16)
            with nc.allow_low_precision(reason="ident casts + consts"):
                nc.gpsimd.tensor_copy(out=identr[:], in_=ident[:])
                nc.gpsimd.tensor_copy(out=identb[:], in_=ident[:])
                nc.gpsimd.tensor_copy(out=wproj_b[:], in_=wproj_f[:])
                # ones column of vaug: row 64 of AV result = softmax denom
                nc.gpsimd.memset(vaug[0][:, :, :, 64], 1.0)
                nc.gpsimd.memset(vaug[1][:, :, :, 64], 1.0)

            out_dsts = [
                OUT[bb].rearrange("(t p) c -> p t c", p=128) for bb in range(NB)
            ]

            # ---------------- unit emitters ----------------
            def transpose_chunk(b, t):
                # x^T via PE transposes; 4 per PSUM bank, single DVE eviction
                pT4 = pmi.tile([128, 4, 128], f32r, tag="px", name="pT4")
                for c4 in range(4):
                    nc.tensor.transpose(
                        pT4[:, c4, :], x_tiles[b][:, t, c4 * 128:(c4 + 1) * 128],
                        identr[:],
                    )
                nc.vector.tensor_copy(
                    out=xT[:, :, t * 128:(t + 1) * 128], in_=pT4[:]
                )

            def qk_unit(b, j, sc):
                # q,k for head pair j, seq half sc -> fp8; q gets a residual
                # term (slot 1), k8 is duplicated into both DoubleRow slots
                p2 = b % 2
                pq = pmi.tile([128, 512], f32, tag="px", name="pq")
                pk = pmi.tile([128, 512], f32, tag="px", name="pk")
                for c4 in range(4):
                    nc.tensor.matmul(
                        pq[:], wq_sb[:, c4, 128 * j:128 * (j + 1)],
                        xT[:, c4, sc * 512:(sc + 1) * 512],
                        start=(c4 == 0), stop=(c4 == 3),
                    )
                for c4 in range(4):
                    nc.tensor.matmul(
                        pk[:], wk_sb[:, c4, 128 * j:128 * (j + 1)],
                        xT[:, c4, sc * 512:(sc + 1) * 512],
                        start=(c4 == 0), stop=(c4 == 3),
                    )
                q0 = qT8[p2][:, j, 0, sc * 512:(sc + 1) * 512]
                q1 = qT8[p2][:, j, 1, sc * 512:(sc + 1) * 512]
                k0 = kT8[p2][:, j, 4 * sc:4 * sc + 4, 0, :]
                k1 = kT8[p2][:, j, 4 * sc:4 * sc + 4, 1, :]
                with nc.allow_low_precision(reason="fp8 quantize"):
                    nc.vector.tensor_copy(out=q0, in_=pq[:])
                    nc.vector.tensor_sub(q1, pq[:], q0)
                    nc.vector.tensor_copy(
                        out=k0, in_=pk[:].rearrange("p (t e) -> p t e", t=4))
                    nc.gpsimd.tensor_copy(out=k1, in_=k0)

            def v_unit(b, t):
                p2 = b % 2
                pv = pmi.tile([128, 512], f32, tag="px", name="pv")
                for c4 in range(4):
                    nc.tensor.matmul(
                        pv[:], xT[:, c4, t * 128:(t + 1) * 128], wv_sb[:, c4, :],
                        start=(c4 == 0), stop=(c4 == 3),
                    )
                with nc.allow_low_precision(reason="bf16 v"):
                    nc.vector.tensor_copy(
                        out=vaug[p2][:, t, :, 0:64],
                        in_=pv[:].rearrange("p (h e) -> p h e", h=H),
                    )

            def scores_half(b, qc, h, g, ptile):
                # 4 fp8 DoubleRow matmuls (k stationary, q moving) + exp
                p2 = b % 2
                bp = 64 * (h % 2)
                j = h // 2
                ps = psc.tile([128, 4, 512], f32, tag="sc", name="ps")
                for i in range(4):
                    kt = 4 * g + i
                    nc.tensor.matmul(
                        ps[:, i, :],
                        kT8[p2][bp:bp + 64, j, kt, :, :],
                        qT8[p2][bp:bp + 64, j, :, qc * 512:(qc + 1) * 512],
                        start=True, stop=True, perf_mode=DR,
                    )
                with nc.allow_low_precision(reason="bf16 probs"):
                    nc.scalar.activation(
                        ptile[:, 4 * g:4 * g + 4, :], ps[:], AF.Exp, scale=SCALE,
                    )

            def av_unit(b, qc, h, ptile, o_nb, rz):
                # transposed AV: probs stationary, v (+ones col) moving;
                # raw evict on DVE, per-q 1/Z normalize on gpsimd
                p2 = b % 2
                pv4 = pav_pool.tile([128, 4, 65], f32, tag="av", name="pav")
                for qt in range(4):
                    for kc in range(8):
                        nc.tensor.matmul(
                            pv4[:, qt, :],
                            ptile[:, kc, qt * 128:(qt + 1) * 128],
                            vaug[p2][:, kc, h, :],
                            start=(kc == 0), stop=(kc == 7),
                        )
                o_raw = sbw.tile([128, 4, 64], bf16, tag="oraw")
                nc.vector.reciprocal(rz[:], pv4[:, :, 64:65])
                with nc.allow_low_precision(reason="bf16 attn out"):
                    nc.vector.tensor_copy(out=o_raw[:], in_=pv4[:, :, 0:64])
                    for qt in range(4):
                        nc.gpsimd.tensor_scalar_mul(
                            o_nb[:, qt, :], o_raw[:, qt, :], rz[:, qt:qt + 1])

            def ot_pair(b, qc, hodd, o_nb_even, o_nb_odd):
                # transpose normalized pair [q,d]->[d,q] into ot
                p2 = b % 2
                j = hodd // 2
                pTo = pmi.tile([128, 4, 128], bf16, tag="px", name="pTo")
                for qt in range(4):
                    nc.tensor.transpose(
                        pTo[0:64, qt, :], o_nb_even[:, qt, :], identb[:])
                    nc.tensor.transpose(
                        pTo[64:128, qt, :], o_nb_odd[:, qt, :], identb[:])
                with nc.allow_low_precision(reason="bf16 ot"):
                    nc.vector.tensor_copy(
                        out=ot[p2][:, j, qc * 512:(qc + 1) * 512],
                        in_=pTo[:],
                    )

            def proj_qb(b, qb, last=False):
                p2 = b % 2
                po = pmi.tile([128, 512], f32, tag="px", name="po")
                for d4 in range(4):
                    nc.tensor.matmul(
                        po[:], ot[p2][:, d4, qb * 128:(qb + 1) * 128],
                        wproj_b[:, d4, :],
                        start=(d4 == 0), stop=(d4 == 3),
                    )
                nc.vector.tensor_copy(out=out_sb[:, qb, :], in_=po[:])
                if last:
                    nc.sync.dma_start(
                        out=out_dsts[b][:, qb:qb + 1, :],
                        in_=out_sb[:, qb:qb + 1, :],
                    )
                elif qb % 2 == 1:
                    nc.sync.dma_start(
                        out=out_dsts[b][:, qb - 1:qb + 1, :],
                        in_=out_sb[:, qb - 1:qb + 1, :],
                    )

            # ---------------- schedule ----------------
            # Ordered queue of filler units with forced emission for data
            # deps: scores(b,qc,h) needs qk(b, h//2, both sc); AV(b) needs
            # all v(b); qk/v(b) read xT written by transposes(b) (xT is
            # single-buffered, so order transposes(b+1) after all qk/v(b)).
            pending = {}
            order = []

            def add_unit(key, thunk):
                pending[key] = thunk
                order.append(key)

            def ensure(key):
                th = pending.pop(key, None)
                if th is not None:
                    order.remove(key)
                    th()

            def pop_extra():
                while order:
                    key = order[0]
                    th = pending.pop(key, None)
                    order.pop(0)
                    if th is not None:
                        th()
                        return True
                return False

            # prologue for batch 0: x^T, V, first q/k pair
            for t in range(8):
                transpose_chunk(0, t)
            for t in range(8):
                v_unit(0, t)
            qk_unit(0, 0, 0)
            qk_unit(0, 0, 1)
            for j in range(1, 4):
                for sc in range(2):
                    add_unit(("qk", 0, j, sc), lambda j=j, sc=sc: qk_unit(0, j, sc))

            it = 0
            prev = None  # (b, qc, h, ptile, o_nb, rz) awaiting AV
            pend_ot = None  # even-head o_nb awaiting pair transpose
            for b in range(NB):
                if b + 1 < NB:
                    for t in range(8):
                        add_unit(("xt", b + 1, t),
                                 lambda b=b, t=t: transpose_chunk(b + 1, t))
                    for t in range(8):
                        add_unit(("v", b + 1, t),
                                 lambda b=b, t=t: v_unit(b + 1, t))
                    for j in range(4):
                        for sc in range(2):
                            add_unit(("qk", b + 1, j, sc),
                                     lambda b=b, j=j, sc=sc: qk_unit(b + 1, j, sc))
                for qc in range(2):
                    for h in range(H):
                        if b > 0 and qc == 0 and h == 0:
                            # force batch-b inputs before first use
                            for t in range(8):
                                ensure(("xt", b, t))
                            for t in range(8):
                                ensure(("v", b, t))
                        ensure(("qk", b, h // 2, 0))
                        ensure(("qk", b, h // 2, 1))
                        ptile = pt[it % 2]
                        scores_half(b, qc, h, 0, ptile)
                        if prev is not None:
                            pb, pqc, ph, ppt, po_nb, prz = prev
                            av_unit(pb, pqc, ph, ppt, po_nb, prz)
                        pop_extra()
                        if h % 2 == 0:
                            pop_extra()
                        scores_half(b, qc, h, 1, ptile)
                        if prev is not None:
                            if ph % 2 == 0:
                                pend_ot = po_nb
                            else:
                                ot_pair(pb, pqc, ph, pend_ot, po_nb)
                                if ph == H - 1:
                                    for qb in range(4 * pqc, 4 * pqc + 4):
                                        add_unit(
                                            ("proj", pb, qb),
                                            lambda pb=pb, qb=qb: proj_qb(
                                                pb, qb, last=(pb == NB - 1)))
                        pop_extra()
                        o_nb = sbo.tile([128, 4, 64], bf16, tag="onb")
                        rz = sbr.tile([128, 4], f32, tag="rz")
                        prev = (b, qc, h, ptile, o_nb, rz)
                        it += 1
            # epilogue: final AV + oT + remaining proj
            pb, pqc, ph, ppt, po_nb, prz = prev
            av_unit(pb, pqc, ph, ppt, po_nb, prz)
            ot_pair(pb, pqc, ph, pend_ot, po_nb)
            for qb in range(4, 8):
                proj_qb(NB - 1, qb, last=True)
            while pop_extra():
                pass

    nc.finalize()
    return nc


def kernel(x, mask, Wqkv, Wproj):
    from concourse.bass_utils import run_bass_kernel_spmd

    if "nc" not in _cache:
        _cache["nc"] = _build()
    nc = _cache["nc"]

    x = np.ascontiguousarray(x, dtype=np.float32)
    Wqkv = np.ascontiguousarray(Wqkv, dtype=np.float32)
    Wproj = np.ascontiguousarray(Wproj, dtype=np.float32)
    in_maps = [
        {"X": x[i * NB:(i + 1) * NB], "WQKV": Wqkv, "WPROJ": Wproj}
        for i in range(NCORES)
    ]
    res = run_bass_kernel_spmd(nc, in_maps, list(range(NCORES)))
    return np.concatenate([r["OUT"] for r in res.results], axis=0)


# revision 13
# speedup vs baseline: 1.2109x; 1.2109x over previous
import sys

if "/root/.axon_site/_ro/trn_rl_repo" not in sys.path:
    sys.path.insert(0, "/root/.axon_site/_ro/trn_rl_repo")

import numpy as np

B, S, D, H, DH = 16, 1024, 512, 8, 64
NCORES = 8
NB = B // NCORES  # batches per core
SCALE = D ** -0.5

_cache = {}


def _build():
    import concourse.bacc as bacc
    import concourse.tile as tile
    import concourse.mybir as mybir
    from concourse.masks import make_identity

    f32 = mybir.dt.float32
    f32r = mybir.dt.float32r
    bf16 = mybir.dt.bfloat16
    fp8 = mybir.dt.float8e4
    AF = mybir.ActivationFunctionType
    DR = mybir.MatmulPerfMode.DoubleRow

    nc = bacc.Bacc("TRN2", target_bir_lowering=False)
    X = nc.declare_dram_parameter("X", [NB, S, D], f32, isOutput=False)
    WQKV = nc.declare_dram_parameter("WQKV", [D, 3 * D], f32, isOutput=False)
    WPROJ = nc.declare_dram_parameter("WPROJ", [D, D], f32, isOutput=False)
    OUT = nc.declare_dram_parameter("OUT", [NB, S, D], f32, isOutput=True)

    with tile.TileContext(nc) as tc:
        with tc.tile_pool(name="sb", bufs=1) as sb, \
             tc.tile_pool(name="sbo", bufs=2) as sbo, \
             tc.tile_pool(name="sbw", bufs=2) as sbw, \
             tc.tile_pool(name="sbr", bufs=2) as sbr, \
             tc.tile_pool(name="pmi", bufs=2, space="PSUM") as pmi, \
             tc.tile_pool(name="psc", bufs=2, space="PSUM") as psc, \
             tc.tile_pool(name="pav", bufs=2, space="PSUM") as pav_pool:
            # persistent SBUF
            wq_sb = sb.tile([128, 4, D], f32r)
            wk_sb = sb.tile([128, 4, D], f32r)
            wv_sb = sb.tile([128, 4, D], f32r)
            wproj_f = sb.tile([128, 4, D], f32)
            wproj_b = sb.tile([128, 4, D], bf16)
            ident = sb.tile([128, 128], f32)
            x_tiles = [sb.tile([128, 8, D], f32r, name=f"x{b}") for b in range(NB)]
            xT = sb.tile([128, 4, S], f32r)
            # double-buffered (by batch parity) attention tensors
            qT8 = [sb.tile([128, 4, 2, S], fp8, name=f"q8_{p}") for p in range(2)]
            kT8 = [sb.tile([128, 4, 8, 2, 128], fp8, name=f"k8_{p}")
                   for p in range(2)]
            vaug = [sb.tile([128, 8, H, 65], bf16, name=f"va_{p}") for p in range(2)]
            pt = [sb.tile([128, 8, 512], bf16, name=f"pt_{p}") for p in range(2)]
            ot = [sb.tile([128, 4, S], bf16, name=f"ot_{p}") for p in range(2)]
            out_sb = sb.tile([128, 8, D], f32)

            # startup DMAs: x halves and q/k weights interleaved on SP (in
            # consumption order); v + proj weights on Pool's queue (idle).
            wqkv_split = WQKV[:].bitcast(f32r).rearrange("(t p) e -> p t e", p=128)
            x0_src = X[0].bitcast(f32r).rearrange("(t p) c -> p t c", p=128)
            # x0 first half per-chunk first: transposes 0-3 gate everything
            for t in range(4):
                nc.sync.dma_start(out=x_tiles[0][:, t:t + 1, :],
                                  in_=x0_src[:, t:t + 1, :])
            for j in range(4):
                for h in (2 * j, 2 * j + 1):
                    nc.sync.dma_start(out=wq_sb[:, :, h * 64:(h + 1) * 64],
                                      in_=wqkv_split[:, :, 192 * h:192 * h + 64])
                for h in (2 * j, 2 * j + 1):
                    nc.sync.dma_start(out=wk_sb[:, :, h * 64:(h + 1) * 64],
                                      in_=wqkv_split[:, :, 192 * h + 64:192 * h + 128])
                if j == 0:
                    nc.sync.dma_start(out=x_tiles[0][:, 4:8, :],
                                      in_=x0_src[:, 4:8, :])
            nc.sync.dma_start(
                out=x_tiles[1][:],
                in_=X[1].bitcast(f32r).rearrange("(t p) c -> p t c", p=128),
            )
            make_identity(nc, ident[:])
            identr = sb.tile([128, 128], f32r)
            identb = sb.tile([128, 128], bf16)
            with nc.allow_low_precision(reason="ident casts + consts"):
                nc.gpsimd.tensor_copy(out=identr[:], in_=ident[:])
                nc.gpsimd.tensor_copy(out=identb[:], in_=ident[:])
                # ones column of vaug: row 64 of AV result = softmax denom
                nc.gpsimd.memset(vaug[0][:, :, :, 64], 1.0)
                nc.gpsimd.memset(vaug[1][:, :, :, 64], 1.0)

            def w_dma_unit():
                # v + proj weights on Pool's queue, deferred so the first
                # k8 dups aren't stuck behind SWDGE descriptor generation
                for h in range(H):
                    nc.gpsimd.dma_start(
                        out=wv_sb[:, :, h * 64:(h + 1) * 64],
                        in_=wqkv_split[:, :, 192 * h + 128:192 * h + 192],
                    )
                nc.gpsimd.dma_start(
                    out=wproj_f[:],
                    in_=WPROJ[:].rearrange("(t p) e -> p t e", p=128),
                )
                with nc.allow_low_precision(reason="bf16 wproj"):
                    nc.gpsimd.tensor_copy(out=wproj_b[:], in_=wproj_f[:])

            out_dsts = [
                OUT[bb].rearrange("(t p) c -> p t c", p=128) for bb in range(NB)
            ]

            # -------- dependency-tracked unit emission --------
            pending = {}
            order = []

            def add_unit(key, thunk):
                pending[key] = thunk
                order.append(key)

            def ensure(key):
                th = pending.pop(key, None)
                if th is not None:
                    order.remove(key)
                    th()

            def pop_extra():
                if order:
                    key = order.pop(0)
                    pending.pop(key)()
                    return True
                return False

            # ---------------- unit emitters ----------------
            def transpose_chunk(b, t):
                # x^T via PE transposes; 4 per PSUM bank, single DVE eviction
                pT4 = pmi.tile([128, 4, 128], f32r, tag="px", name="pT4")
                for c4 in range(4):
                    nc.tensor.transpose(
                        pT4[:, c4, :], x_tiles[b][:, t, c4 * 128:(c4 + 1) * 128],
                        identr[:],
                    )
                nc.vector.tensor_copy(
                    out=xT[:, :, t * 128:(t + 1) * 128], in_=pT4[:]
                )

            def q_unit(b, j, sc):
                # q for head pair j, seq half sc -> fp8 + fp8 residual slot
                for t in range(4 * sc, 4 * sc + 4):
                    ensure(("xt", b, t))
                p2 = b % 2
                pq = pmi.tile([128, 512], f32, tag="px", name="pq")
                for c4 in range(4):
                    nc.tensor.matmul(
                        pq[:], wq_sb[:, c4, 128 * j:128 * (j + 1)],
                        xT[:, c4, sc * 512:(sc + 1) * 512],
                        start=(c4 == 0), stop=(c4 == 3),
                    )
                q0 = qT8[p2][:, j, 0, sc * 512:(sc + 1) * 512]
                q1 = qT8[p2][:, j, 1, sc * 512:(sc + 1) * 512]
                with nc.allow_low_precision(reason="fp8 quantize"):
                    nc.vector.tensor_copy(out=q0, in_=pq[:])
                    nc.vector.tensor_sub(q1, pq[:], q0)

            def k_unit(b, j, sc):
                # k8 duplicated into both DoubleRow slots (dup via gpsimd)
                for t in range(4 * sc, 4 * sc + 4):
                    ensure(("xt", b, t))
                p2 = b % 2
                pk = pmi.tile([128, 512], f32, tag="px", name="pk")
                for c4 in range(4):
                    nc.tensor.matmul(
                        pk[:], wk_sb[:, c4, 128 * j:128 * (j + 1)],
                        xT[:, c4, sc * 512:(sc + 1) * 512],
                        start=(c4 == 0), stop=(c4 == 3),
                    )
                k0 = kT8[p2][:, j, 4 * sc:4 * sc + 4, 0, :]
                k1 = kT8[p2][:, j, 4 * sc:4 * sc + 4, 1, :]
                with nc.allow_low_precision(reason="fp8 quantize"):
                    nc.vector.tensor_copy(
                        out=k0, in_=pk[:].rearrange("p (t e) -> p t e", t=4))
                    nc.gpsimd.tensor_copy(out=k1, in_=k0)

            def v_unit(b, t):
                ensure(("wdma",))
                ensure(("xt", b, t))
                p2 = b % 2
                pv = pmi.tile([128, 512], f32, tag="px", name="pv")
                for c4 in range(4):
                    nc.tensor.matmul(
                        pv[:], xT[:, c4, t * 128:(t + 1) * 128], wv_sb[:, c4, :],
                        start=(c4 == 0), stop=(c4 == 3),
                    )
                with nc.allow_low_precision(reason="bf16 v"):
                    nc.vector.tensor_copy(
                        out=vaug[p2][:, t, :, 0:64],
                        in_=pv[:].rearrange("p (h e) -> p h e", h=H),
                    )

            def scores_quarter(b, qc, h, g, ptile):
                # 2 fp8 DoubleRow matmuls (k stationary, q moving) + exp
                p2 = b % 2
                bp = 64 * (h % 2)
                j = h // 2
                ps = psc.tile([128, 2, 512], f32, tag="sc", name="ps")
                for i in range(2):
                    kt = 2 * g + i
                    nc.tensor.matmul(
                        ps[:, i, :],
                        kT8[p2][bp:bp + 64, j, kt, :, :],
                        qT8[p2][bp:bp + 64, j, :, qc * 512:(qc + 1) * 512],
                        start=True, stop=True, perf_mode=DR,
                    )
                with nc.allow_low_precision(reason="bf16 probs"):
                    nc.scalar.activation(
                        ptile[:, 2 * g:2 * g + 2, :], ps[:], AF.Exp, scale=SCALE,
                    )

            def av_unit(b, qc, h, ptile, o_nb, rz):
                # transposed AV: probs stationary, v (+ones col) moving;
                # raw evict on DVE, per-q 1/Z normalize on gpsimd
                p2 = b % 2
                pv4 = pav_pool.tile([128, 4, 65], f32, tag="av", name="pav")
                for qt in range(4):
                    for kc in range(8):
                        nc.tensor.matmul(
                            pv4[:, qt, :],
                            ptile[:, kc, qt * 128:(qt + 1) * 128],
                            vaug[p2][:, kc, h, :],
                            start=(kc == 0), stop=(kc == 7),
                        )
                o_raw = sbw.tile([128, 4, 64], bf16, tag="oraw")
                nc.vector.reciprocal(rz[:], pv4[:, :, 64:65])
                with nc.allow_low_precision(reason="bf16 attn out"):
                    nc.vector.tensor_copy(out=o_raw[:], in_=pv4[:, :, 0:64])
                    for qt in range(4):
                        nc.gpsimd.tensor_scalar_mul(
                            o_nb[:, qt, :], o_raw[:, qt, :], rz[:, qt:qt + 1])

            def ot_pair(b, qc, hodd, o_nb_even, o_nb_odd):
                # transpose normalized pair [q,d]->[d,q] into ot
                p2 = b % 2
                j = hodd // 2
                pTo = pmi.tile([128, 4, 128], bf16, tag="px", name="pTo")
                for qt in range(4):
                    nc.tensor.transpose(
                        pTo[0:64, qt, :], o_nb_even[:, qt, :], identb[:])
                    nc.tensor.transpose(
                        pTo[64:128, qt, :], o_nb_odd[:, qt, :], identb[:])
                with nc.allow_low_precision(reason="bf16 ot"):
                    nc.vector.tensor_copy(
                        out=ot[p2][:, j, qc * 512:(qc + 1) * 512],
                        in_=pTo[:],
                    )

            def proj_qb(b, qb, last=False):
                p2 = b % 2
                po = pmi.tile([128, 512], f32, tag="px", name="po")
                for d4 in range(4):
                    nc.tensor.matmul(
                        po[:], ot[p2][:, d4, qb * 128:(qb + 1) * 128],
                        wproj_b[:, d4, :],
                        start=(d4 == 0), stop=(d4 == 3),
                    )
                nc.vector.tensor_copy(out=out_sb[:, qb, :], in_=po[:])
                if last:
                    nc.sync.dma_start(
                        out=out_dsts[b][:, qb:qb + 1, :],
                        in_=out_sb[:, qb:qb + 1, :],
                    )
                elif qb % 2 == 1:
                    nc.sync.dma_start(
                        out=out_dsts[b][:, qb - 1:qb + 1, :],
                        in_=out_sb[:, qb - 1:qb + 1, :],
                    )

            # ---------------- schedule ----------------
            for bb in range(NB):
                for t in range(8):
                    add_unit(("xt", bb, t), lambda b=bb, t=t: transpose_chunk(b, t))
                for j in range(4):
                    for sc in range(2):
                        add_unit(("q", bb, j, sc), lambda b=bb, j=j, sc=sc: q_unit(b, j, sc))
                        add_unit(("k", bb, j, sc), lambda b=bb, j=j, sc=sc: k_unit(b, j, sc))
                if bb == 0:
                    add_unit(("wdma",), w_dma_unit)
                for t in range(8):
                    add_unit(("v", bb, t), lambda b=bb, t=t: v_unit(b, t))

            it = 0
            prev = None  # (b, qc, h, ptile, o_nb, rz) awaiting AV
            pend_ot = None  # even-head o_nb awaiting pair transpose
            for b in range(NB):
                for qc in range(2):
                    for h in range(H):
                        j = h // 2
                        ensure(("q", b, j, qc))
                        ensure(("k", b, j, 0))
                        if prev is not None and prev[0] == b and prev[1] == 0 \
                                and prev[2] == 0:
                            for t in range(8):
                                ensure(("v", b, t))
                        ptile = pt[it % 2]
                        scores_quarter(b, qc, h, 0, ptile)
                        scores_quarter(b, qc, h, 1, ptile)
                        ensure(("k", b, j, 1))
                        if prev is not None:
                            pb, pqc, ph, ppt, po_nb, prz = prev
                            av_unit(pb, pqc, ph, ppt, po_nb, prz)
                        scores_quarter(b, qc, h, 2, ptile)
                        pop_extra()
                        scores_quarter(b, qc, h, 3, ptile)
                        if prev is not None:
                            if ph % 2 == 0:
                                pend_ot = po_nb
                            else:
                                ot_pair(pb, pqc, ph, pend_ot, po_nb)
                                if ph == H - 1:
                                    for qb in range(4 * pqc, 4 * pqc + 4):
                                        add_unit(
                                            ("proj", pb, qb),
                                            lambda pb=pb, qb=qb: proj_qb(
                                                pb, qb, last=(pb == NB - 1)))
                        pop_extra()
                        if h % 2 == 1:
                            pop_extra()
                        if b + 1 < NB and qc == 1 and h == 5:
                            # prime next batch's first inputs before boundary
                            ensure(("q", b + 1, 0, 0))
                            ensure(("k", b + 1, 0, 0))
                            ensure(("k", b + 1, 0, 1))
                        o_nb = sbo.tile([128, 4, 64], bf16, tag="onb")
                        rz = sbr.tile([128, 4], f32, tag="rz")
                        prev = (b, qc, h, ptile, o_nb, rz)
                        it += 1
            # epilogue: final AV + oT + remaining proj
            pb, pqc, ph, ppt, po_nb, prz = prev
            av_unit(pb, pqc, ph, ppt, po_nb, prz)
            ot_pair(pb, pqc, ph, pend_ot, po_nb)
            for qb in range(4, 8):
                proj_qb(NB - 1, qb, last=True)
            while pop_extra():
                pass

    nc.finalize()
    return nc


def kernel(x, mask, Wqkv, Wproj):
    from concourse.bass_utils import run_bass_kernel_spmd

    if "nc" not in _cache:
        _cache["nc"] = _build()
    nc = _cache["nc"]

    x = np.ascontiguousarray(x, dtype=np.float32)
    Wqkv = np.ascontiguousarray(Wqkv, dtype=np.float32)
    Wproj = np.ascontiguousarray(Wproj, dtype=np.float32)
    in_maps = [
        {"X": x[i * NB:(i + 1) * NB], "WQKV": Wqkv, "WPROJ": Wproj}
        for i in range(NCORES)
    ]
    res = run_bass_kernel_spmd(nc, in_maps, list(range(NCORES)))
    return np.concatenate([r["OUT"] for r in res.results], axis=0)
